# revision 34
# baseline (speedup 1.0000x reference)
"""Single-head attention (B=4, T=4096, C=1024, H=64) on 8 trn2 NeuronCores.

Sharding: 8 shards = (batch b, query-half h).  Each core receives x[b]
pre-transposed to xT [C=1024, T=4096] in bf16; for h==1 the T columns are
rotated by 2048 so "this core's" 2048 queries are always columns 0:2048
(softmax is permutation-invariant over keys).  SPMD: identical program on
every core.

Per-core kernel (flash-attention style, all matmuls bf16):

phase 1 (projections): stream xT in [128,1024] bf16 tiles (8 c-chunks per
  1024-column block "sbb").  Two stationary streams:
    A: even 512-blocks [Wk|Wv], odd 512-blocks [Wv|Wk]  ->  kv2 [128, T]
       (K^T lives at partitions 0:64 for even blocks, 64:128 for odd ones;
        V^T on the other half) -- this feeds the row-tiled scores matmul
        with NO replication copies.
    B: [Wq|Wq] (query blocks only)  ->  qq [128, TQ]: Q^T replicated into
       both partition halves (the moving operand of both score row-tiles).
  V tiles are PE-transposed out of kv2 into va [s,64]+ones column (so the
  softmax denominator falls out of the attn@V matmul).

phase 2 (attention), per 1024-query group, per pair of key tiles (one from
  an even 512-block at partitions 0:64, one from an odd one at 64:128):
    PE: TWO CONCURRENT K=64 row-tiled matmuls (tile_position (0,0)/(64,0))
        compute scoresT [128 keys, 1024 q] for both tiles in the time of one.
    exp: tile a -> ACT true exp (bf16 out); tile b -> DVE Schraudolph
        bit-trick exp (i16 = s*a + b, bitcast bf16), splitting the exp wall
        across two engines.
    PE: outT[65, 512] += va[s,65].T @ ex  (accumulated over all 32 tiles).
  Epilogue: PE-transpose outT back to [q, 65], DVE reciprocal of the
  ones-column sum, scale, DMA out.
"""

import os
import sys

for _p in ("/opt/trn_rl_repo", "/root/.axon_site/_ro/trn_rl_repo"):
    if os.path.isdir(_p) and _p not in sys.path:
        sys.path.append(_p)

import numpy as np
import ml_dtypes

import concourse.bacc as bacc
import concourse.mybir as mybir
import concourse.tile as tile
from concourse.bass_utils import run_bass_kernel_spmd
from concourse.masks import make_identity

B = 4
T = 4096
C = 1024
H = 64
TQ = T // 2  # queries per core
N_CORES = 8

F32 = mybir.dt.float32
BF16 = mybir.dt.bfloat16
I16 = mybir.dt.int16

NC_CH = C // 128  # 8 contraction chunks
NSBB = T // 1024  # 4 1024-wide column blocks
NST = T // 128  # 32 key tiles of 128
NPAIR = NST // 2  # 16 row-tiled score pairs

EXP = mybir.ActivationFunctionType.Exp

# Schraudolph exp in bf16: i16 = trunc(z * 2^7/ln2 + (127*2^7 - c)),
# bitcast to bf16.  c = 0.0436775*128 - 0.5 (the -0.5 compensates
# truncation vs round-to-nearest).  The 0.125 score scale is folded in.
SCHRA_A = 0.125 * (2.0**7) / np.log(2.0)
SCHRA_B = 127.0 * 2.0**7 - (0.0436775 * 2.0**7 - 0.5)


def _build_module():
    nc = bacc.Bacc("TRN2", target_bir_lowering=False, debug=False, num_devices=N_CORES)

    xT = nc.dram_tensor("xT", [NSBB, NC_CH, 128, 1024], BF16, kind="ExternalInput").ap()
    wkv = nc.dram_tensor("wkv", [128, NC_CH, 128], BF16, kind="ExternalInput").ap()
    wvk = nc.dram_tensor("wvk", [128, NC_CH, 128], BF16, kind="ExternalInput").ap()
    wqq = nc.dram_tensor("wqq", [128, NC_CH, 128], BF16, kind="ExternalInput").ap()
    out = nc.dram_tensor("out", [TQ, H], F32, kind="ExternalOutput").ap()

    with tile.TileContext(nc) as tc:
        with (
            tc.tile_pool(name="const", bufs=1) as const_pool,
            tc.tile_pool(name="xt", bufs=32) as xt_pool,
            tc.tile_pool(name="big", bufs=1) as big_pool,
            tc.tile_pool(name="exp", bufs=16) as exp_pool,
            tc.tile_pool(name="outts", bufs=2) as outts_pool,
            tc.tile_pool(name="small", bufs=4) as small_pool,
            # one unified 6-slot ring (slot = 2KB/partition = 1 bank) for
            # proj / transposes / scores / epilogue; +2 banks accumulators.
            # Each score chunk gets its OWN single-bank tile: ACT and DVE can
            # only access PSUM in parallel on different banks, so a 2-bank
            # tile read by both engines would serialize them.
            tc.tile_pool(name="ps", bufs=6, space="PSUM") as psum_ps,
            tc.tile_pool(name="pacc", bufs=2, space="PSUM") as psum_acc,
        ):
            # ---- constants ----
            wkv_sb = const_pool.tile([128, NC_CH, 128], BF16, tag="wkv")
            wvk_sb = const_pool.tile([128, NC_CH, 128], BF16, tag="wvk")
            wqq_sb = const_pool.tile([128, NC_CH, 128], BF16, tag="wqq")
            ident_bf = const_pool.tile([128, 128], BF16, tag="ident_bf")
            ident_f32 = const_pool.tile([128, 128], F32, tag="ident_f32")
            # HAM warmup: junk matmuls while the x DMA streams in, so the
            # PE clock gate releases (1.2 -> 2.4 GHz) before real work starts.
            scratch = const_pool.tile([128, 512], BF16, tag="scratch")
            nc.gpsimd.memset(scratch[:], 0.0)
            for w in range(8):
                wps = psum_ps.tile([128, 512], F32, tag="ps")
                nc.tensor.matmul(
                    wps[:], scratch[:, 0:128], scratch[:], start=True, stop=True
                )

            nc.scalar.dma_start(wkv_sb[:], wkv[:])
            nc.scalar.dma_start(wvk_sb[:], wvk[:])
            nc.scalar.dma_start(wqq_sb[:], wqq[:])
            make_identity(nc, ident_bf[:])
            make_identity(nc, ident_f32[:])

            # warm the ACT exp table early (one-time ~2.7us load)
            dummy = small_pool.tile([128, 1], F32, tag="dummy")
            nc.scalar.activation(dummy[:], ident_f32[:, 0:1], EXP)

            # ---- persistent activations ----
            # kv2: even 512-block: rows 0:64 K^T, 64:128 V^T; odd: swapped
            kv2 = big_pool.tile([128, T], BF16, tag="kv2")
            qq = big_pool.tile([128, TQ], BF16, tag="qq")  # Q^T replicated
            va = big_pool.tile([128, NST, 66], BF16, tag="va")  # V[s,64]+ones
            nc.gpsimd.memset(va[:, :, 64:65], 1.0)

            # ---- x DMA: issue everything up front, 3 trigger engines ----
            xts = {}  # (sbb, c) -> tile
            for sbb in range(NSBB):
                for c in range(NC_CH):
                    xts[(sbb, c)] = xt_pool.tile(
                        [128, 1024], BF16, tag="xt", name=f"xt{sbb}_{c}"
                    )
            # Strictly consumption-ordered triggers on two engines (c 0-3 on
            # sync, c 4-7 on gpsimd).  Issuing everything up front would put
            # 30+ transfers in flight at once and starve the FIRST tile of
            # HBM bandwidth (all queues share ~358 GB/s).
            for c in range(NC_CH):  # sbb0 split in halves on two engines
                xt = xts[(0, c)]
                eng = nc.sync if c < 4 else nc.gpsimd
                eng.dma_start(xt[:, 0:512], xT[0, c, :, 0:512])
                eng.dma_start(xt[:, 512:1024], xT[0, c, :, 512:1024])
            trig = (nc.sync, nc.gpsimd, nc.scalar)
            for sbb in (1, 2, 3):
                for c in range(NC_CH):
                    trig[c % 3].dma_start(xts[(sbb, c)][:], xT[sbb, c])

            # ---- phase 1: projections for one 1024-col block ----
            def emit_proj_block(sbb):
                is_q = sbb < 2
                for half in range(2):
                    sb = 2 * sbb + half
                    wa = wkv_sb if half == 0 else wvk_sb
                    cols = slice(sb * 512, (sb + 1) * 512)
                    xsl = slice(half * 512, (half + 1) * 512)
                    ps_a = psum_ps.tile([128, 512], F32, tag="ps")
                    for c in range(NC_CH):
                        nc.tensor.matmul(
                            ps_a[:],
                            wa[:, c, :],
                            xts[(sbb, c)][:, xsl],
                            start=(c == 0),
                            stop=(c == NC_CH - 1),
                        )
                    nc.scalar.copy(kv2[:, cols], ps_a[:])
                    if is_q:
                        ps_b = psum_ps.tile([128, 512], F32, tag="ps")
                        for c in range(NC_CH):
                            nc.tensor.matmul(
                                ps_b[:],
                                wqq_sb[:, c, :],
                                xts[(sbb, c)][:, xsl],
                                start=(c == 0),
                                stop=(c == NC_CH - 1),
                            )
                        nc.scalar.copy(qq[:, cols], ps_b[:])
                    # V tiles of this 512-block -> va (PE transpose)
                    vrows = slice(64, 128) if half == 0 else slice(0, 64)
                    tp_pos = (64, 0) if half == 0 else (0, 0)
                    ident_sl = (
                        ident_bf[64:128, 64:128] if half == 0 else ident_bf[0:64, 0:64]
                    )
                    for j in range(4):
                        st = sb * 4 + j
                        tp = psum_ps.tile([128, 64], BF16, tag="ps")
                        nc.tensor.transpose(
                            tp[:],
                            kv2[vrows, st * 128 : (st + 1) * 128],
                            ident_sl,
                            tile_position=tp_pos,
                        )
                        nc.vector.tensor_copy(va[:, st, 0:64], tp[:])

            # key-tile pairs: (tile in even 512-block, tile in odd one)
            pairs = []
            for g in range(NSBB):
                for i in range(4):
                    pairs.append((g * 8 + i, g * 8 + 4 + i))

            # ---- phase 2 ----
            acc_tiles = {}

            def emit_scores(tcp, j):
                ta, tb = pairs[j]
                tc0 = tcp * 1024
                sc = {}
                for i in range(2):
                    qsl = slice(tc0 + i * 512, tc0 + (i + 1) * 512)
                    sc_a = psum_ps.tile(
                        [128, 512], F32, tag="ps", name=f"sca{i}_{tcp}_{j}"
                    )
                    sc_b = psum_ps.tile(
                        [128, 512], F32, tag="ps", name=f"scb{i}_{tcp}_{j}"
                    )
                    sc[("a", i)], sc[("b", i)] = sc_a, sc_b
                    nc.tensor.matmul(
                        sc_a[:],
                        kv2[0:64, ta * 128 : (ta + 1) * 128],
                        qq[0:64, qsl],
                        start=True,
                        stop=True,
                        tile_position=(0, 0),
                    )
                    nc.tensor.matmul(
                        sc_b[:],
                        kv2[64:128, tb * 128 : (tb + 1) * 128],
                        qq[64:128, qsl],
                        start=True,
                        stop=True,
                        tile_position=(64, 0),
                    )
                # exp, split per 512-chunk across engines: ACT takes the c0
                # halves (true exp), DVE the c1 halves (Schraudolph bit
                # trick) -- each query column uses one consistent method and
                # the sc psum slot is released sooner than one [128,1024]
                # pass.  Each ex tile has a SINGLE writer (Tile syncs WAW at
                # tile granularity across engines, so a shared tile would
                # ping-pong serialize ACT and DVE).
                def act_exp(sct, nm):
                    ex = exp_pool.tile([128, 512], BF16, tag="exp", name=nm)
                    nc.scalar.activation(ex[:], sct[:], EXP, scale=0.125)
                    return ex

                def dve_exp(sct, nm):
                    ex = exp_pool.tile([128, 512], BF16, tag="exp", name=nm)
                    nc.vector.tensor_scalar(
                        ex[:].bitcast(I16),
                        sct[:],
                        SCHRA_A,
                        SCHRA_B,
                        mybir.AluOpType.mult,
                        mybir.AluOpType.add,
                    )
                    return ex

                ex_a = (
                    act_exp(sc[("a", 0)], f"exa0_{tcp}_{j}"),
                    dve_exp(sc[("a", 1)], f"exa1_{tcp}_{j}"),
                )
                ex_b = (
                    act_exp(sc[("b", 0)], f"exb0_{tcp}_{j}"),
                    dve_exp(sc[("b", 1)], f"exb1_{tcp}_{j}"),
                )
                return ex_a, ex_b

            def emit_av(tcp, j, ex_a, ex_b):
                ta, tb = pairs[j]
                oc0, oc1 = acc_tiles[tcp]
                for t, ex in ((ta, ex_a), (tb, ex_b)):
                    first = j == 0 and t == ta
                    last = j == NPAIR - 1 and t == tb
                    for i, oc in enumerate((oc0, oc1)):
                        nc.tensor.matmul(
                            oc[:],
                            va[:, t, 0:65],
                            ex[i][:],
                            start=first,
                            stop=last,
                        )

            def emit_attn(tcp, j_lo, j_hi):
                if tcp not in acc_tiles:
                    acc_tiles[tcp] = (
                        psum_acc.tile([65, 512], F32, tag="acc", name=f"oc0_{tcp}"),
                        psum_acc.tile([65, 512], F32, tag="acc", name=f"oc1_{tcp}"),
                    )
                pend = []  # scores run one pair ahead of attn@V
                for j in range(j_lo, j_hi):
                    pend.append((j, emit_scores(tcp, j)))
                    if len(pend) > 1:
                        pj, (pa, pb) = pend.pop(0)
                        emit_av(tcp, pj, pa, pb)
                for pj, (pa, pb) in pend:
                    emit_av(tcp, pj, pa, pb)

            def emit_epilogue_copies(tcp):
                # frees the psum accumulators (the only thing the next query
                # group's attn@V waits on), so the rest of the epilogue can
                # trail into the next group.
                oc0, oc1 = acc_tiles[tcp]
                outt_sb = outts_pool.tile(
                    [65, 1024], F32, tag="outts", name=f"outts{tcp}"
                )
                nc.scalar.copy(outt_sb[:, 0:512], oc0[:])
                nc.scalar.copy(outt_sb[:, 512:1024], oc1[:])
                return outt_sb

            COPY = mybir.ActivationFunctionType.Copy
            out_dma_engines = (nc.sync, nc.scalar, nc.gpsimd)

            def emit_epilogue(tcp, outt_sb):
                # per 128-query block: PE transpose, DVE reciprocal of the
                # ones-column, ACT scaled-copy (out = outT.T * 1/den), DMA.
                # Normalize work is split across DVE+ACT and the out-DMA
                # triggers across three engines so no single queue serializes
                # the tail.
                for k in range(8):
                    o_ps = psum_ps.tile([128, 65], F32, tag="ps")
                    nc.tensor.transpose(
                        o_ps[:],
                        outt_sb[0:65, k * 128 : (k + 1) * 128],
                        ident_f32[0:65, 0:65],
                    )
                    rc = small_pool.tile([128, 1], F32, tag="rc")
                    nc.vector.reciprocal(rc[:], o_ps[:, 64:65])
                    o_sb = small_pool.tile([128, H], F32, tag="osb")
                    nc.scalar.activation(o_sb[:], o_ps[:, 0:H], COPY, scale=rc[:])
                    row = tcp * 1024 + k * 128
                    out_dma_engines[k % 3].dma_start(out[row : row + 128, :], o_sb[:])

            # ---- emission order ----
            # All projection work first; the attention stream is pushed
            # behind it with a manual schedule timestamp so the scheduler
            # cannot interleave late proj matmuls/transposes into the
            # (weight-pipelined) scores/attn@V streams.
            # Alternate proj blocks with tcp0 attention chunks: pair group g
            # (4 pairs) only touches block g's tiles, so each attention chunk
            # fills the DMA-wait gap of the NEXT proj block.  Emission order
            # = scheduler priority, so later proj blocks cannot preempt the
            # already-emitted attention stream.
            emit_proj_block(0)
            emit_attn(0, 0, 4)
            emit_proj_block(1)
            emit_attn(0, 4, 8)
            emit_proj_block(2)
            emit_attn(0, 8, 12)
            emit_proj_block(3)
            emit_attn(0, 12, NPAIR)
            outts0 = emit_epilogue_copies(0)
            emit_attn(1, 0, NPAIR)
            emit_epilogue(0, outts0)
            outts1 = emit_epilogue_copies(1)
            emit_epilogue(1, outts1)

    nc.compile()
    return nc


_NC_CACHE = None


def _get_module():
    global _NC_CACHE
    if _NC_CACHE is None:
        _NC_CACHE = _build_module()
    return _NC_CACHE


def _make_in_maps(x, Wq, Wk, Wv):
    bf = ml_dtypes.bfloat16
    xT = np.transpose(np.asarray(x, dtype=np.float32), (0, 2, 1))  # [B, C, T]
    wq = np.asarray(Wq, dtype=np.float32)
    wk = np.asarray(Wk, dtype=np.float32)
    wv = np.asarray(Wv, dtype=np.float32)

    def wprep(a, b):
        # [C, 128] -> sbuf layout [128 partitions, NC_CH, 128]
        w = np.concatenate([a, b], axis=1).reshape(NC_CH, 128, 128)
        return np.ascontiguousarray(w.transpose(1, 0, 2)).astype(bf)

    wkv = wprep(wk, wv)
    wvk = wprep(wv, wk)
    wqq = wprep(wq, wq)
    in_maps = []
    for core in range(N_CORES):
        b, h = divmod(core, 2)
        xt = xT[b]
        if h == 1:
            xt = np.concatenate([xt[:, TQ:], xt[:, :TQ]], axis=1)
        xt = np.ascontiguousarray(
            xt.reshape(NC_CH, 128, NSBB, 1024).transpose(2, 0, 1, 3)
        ).astype(bf)
        in_maps.append({"xT": xt, "wkv": wkv, "wvk": wvk, "wqq": wqq})
    return in_maps


def run(x, Wq, Wk, Wv, **spmd_kwargs):
    """Run on hardware; returns (output, BassKernelResults)."""
    nc = _get_module()
    in_maps = _make_in_maps(x, Wq, Wk, Wv)
    res = run_bass_kernel_spmd(nc, in_maps, core_ids=list(range(N_CORES)), **spmd_kwargs)
    out = np.empty((B, T, H), dtype=np.float32)
    for core in range(N_CORES):
        b, h = divmod(core, 2)
        out[b, h * TQ : (h + 1) * TQ, :] = res.results[core]["out"]
    return out, res


def kernel(x, Wq, Wk, Wv):
    out, _ = run(x, Wq, Wk, Wv)
    return out


# revision 35
# speedup vs baseline: 1.0326x; 1.0326x over previous
"""Single-head attention (B=4, T=4096, C=1024, H=64) on 8 trn2 NeuronCores.

Sharding: 8 shards = (batch b, query-half h).  Each core receives x[b]
pre-transposed to xT [C=1024, T=4096] in bf16; for h==1 the T columns are
rotated by 2048 so "this core's" 2048 queries are always columns 0:2048
(softmax is permutation-invariant over keys).  SPMD: identical program on
every core.

Per-core kernel (flash-attention style, all matmuls bf16):

phase 1 (projections): stream xT in [128,1024] bf16 tiles (8 c-chunks per
  1024-column block "sbb").  Two stationary streams:
    A: even 512-blocks [Wk|Wv], odd 512-blocks [Wv|Wk]  ->  kv2 [128, T]
       (K^T lives at partitions 0:64 for even blocks, 64:128 for odd ones;
        V^T on the other half) -- this feeds the row-tiled scores matmul
        with NO replication copies.
    B: [Wq|Wq] (query blocks only)  ->  qq [128, TQ]: Q^T replicated into
       both partition halves (the moving operand of both score row-tiles).
  V tiles are PE-transposed out of kv2 into va [s,64]+ones column (so the
  softmax denominator falls out of the attn@V matmul).

phase 2 (attention), per 1024-query group, per pair of key tiles (one from
  an even 512-block at partitions 0:64, one from an odd one at 64:128):
    PE: TWO CONCURRENT K=64 row-tiled matmuls (tile_position (0,0)/(64,0))
        compute scoresT [128 keys, 1024 q] for both tiles in the time of one.
    exp: tile a -> ACT true exp (bf16 out); tile b -> DVE Schraudolph
        bit-trick exp (i16 = s*a + b, bitcast bf16), splitting the exp wall
        across two engines.
    PE: outT[65, 512] += va[s,65].T @ ex  (accumulated over all 32 tiles).
  Epilogue: PE-transpose outT back to [q, 65], DVE reciprocal of the
  ones-column sum, scale, DMA out.
"""

import os
import sys

for _p in ("/opt/trn_rl_repo", "/root/.axon_site/_ro/trn_rl_repo"):
    if os.path.isdir(_p) and _p not in sys.path:
        sys.path.append(_p)

import numpy as np
import ml_dtypes

import concourse.bacc as bacc
import concourse.mybir as mybir
import concourse.tile as tile
from concourse.bass_utils import run_bass_kernel_spmd
from concourse.masks import make_identity

B = 4
T = 4096
C = 1024
H = 64
TQ = T // 2  # queries per core
N_CORES = 8

F32 = mybir.dt.float32
BF16 = mybir.dt.bfloat16
I16 = mybir.dt.int16

NC_CH = C // 128  # 8 contraction chunks
NSBB = T // 1024  # 4 1024-wide column blocks
NST = T // 128  # 32 key tiles of 128
NPAIR = NST // 2  # 16 row-tiled score pairs

EXP = mybir.ActivationFunctionType.Exp

# Schraudolph exp in bf16: i16 = trunc(z * 2^7/ln2 + (127*2^7 - c)),
# bitcast to bf16.  c = 0.0436775*128 - 0.5 (the -0.5 compensates
# truncation vs round-to-nearest).  The 0.125 score scale is folded in.
SCHRA_A = 0.125 * (2.0**7) / np.log(2.0)
SCHRA_B = 127.0 * 2.0**7 - (0.0436775 * 2.0**7 - 0.5)


def _build_module():
    nc = bacc.Bacc("TRN2", target_bir_lowering=False, debug=False, num_devices=N_CORES)

    xT = nc.dram_tensor("xT", [NSBB, NC_CH, 128, 1024], BF16, kind="ExternalInput").ap()
    wkv = nc.dram_tensor("wkv", [128, NC_CH, 128], BF16, kind="ExternalInput").ap()
    wvk = nc.dram_tensor("wvk", [128, NC_CH, 128], BF16, kind="ExternalInput").ap()
    wqq = nc.dram_tensor("wqq", [128, NC_CH, 128], BF16, kind="ExternalInput").ap()
    out = nc.dram_tensor("out", [TQ, H], F32, kind="ExternalOutput").ap()

    with tile.TileContext(nc) as tc:
        with (
            tc.tile_pool(name="const", bufs=1) as const_pool,
            tc.tile_pool(name="xt", bufs=32) as xt_pool,
            tc.tile_pool(name="big", bufs=1) as big_pool,
            tc.tile_pool(name="exp", bufs=16) as exp_pool,
            tc.tile_pool(name="outts", bufs=2) as outts_pool,
            tc.tile_pool(name="small", bufs=4) as small_pool,
            # one unified 6-slot ring (slot = 2KB/partition = 1 bank) for
            # proj / transposes / scores / epilogue; +2 banks accumulators.
            # Each score chunk gets its OWN single-bank tile: ACT and DVE can
            # only access PSUM in parallel on different banks, so a 2-bank
            # tile read by both engines would serialize them.
            tc.tile_pool(name="ps", bufs=6, space="PSUM") as psum_ps,
            tc.tile_pool(name="pacc", bufs=2, space="PSUM") as psum_acc,
        ):
            # ---- constants ----
            wkv_sb = const_pool.tile([128, NC_CH, 128], BF16, tag="wkv")
            wvk_sb = const_pool.tile([128, NC_CH, 128], BF16, tag="wvk")
            wqq_sb = const_pool.tile([128, NC_CH, 128], BF16, tag="wqq")
            ident_bf = const_pool.tile([128, 128], BF16, tag="ident_bf")
            ident_f32 = const_pool.tile([128, 128], F32, tag="ident_f32")
            # HAM warmup: junk matmuls while the x DMA streams in, so the
            # PE clock gate releases (1.2 -> 2.4 GHz) before real work starts.
            scratch = const_pool.tile([128, 512], BF16, tag="scratch")
            nc.gpsimd.memset(scratch[:], 0.0)
            for w in range(8):
                wps = psum_ps.tile([128, 512], F32, tag="ps")
                nc.tensor.matmul(
                    wps[:], scratch[:, 0:128], scratch[:], start=True, stop=True
                )

            nc.scalar.dma_start(wkv_sb[:], wkv[:])
            nc.scalar.dma_start(wvk_sb[:], wvk[:])
            nc.scalar.dma_start(wqq_sb[:], wqq[:])
            make_identity(nc, ident_bf[:])
            make_identity(nc, ident_f32[:])

            # warm the ACT exp table early (one-time ~2.7us load)
            dummy = small_pool.tile([128, 1], F32, tag="dummy")
            nc.scalar.activation(dummy[:], ident_f32[:, 0:1], EXP)

            # ---- persistent activations ----
            # kv2: even 512-block: rows 0:64 K^T, 64:128 V^T; odd: swapped
            kv2 = big_pool.tile([128, T], BF16, tag="kv2")
            qq = big_pool.tile([128, TQ], BF16, tag="qq")  # Q^T replicated
            va = big_pool.tile([128, NST, 66], BF16, tag="va")  # V[s,64]+ones
            nc.gpsimd.memset(va[:, :, 64:65], 1.0)

            # ---- x DMA: issue everything up front, 3 trigger engines ----
            xts = {}  # (sbb, c) -> tile
            for sbb in range(NSBB):
                for c in range(NC_CH):
                    xts[(sbb, c)] = xt_pool.tile(
                        [128, 1024], BF16, tag="xt", name=f"xt{sbb}_{c}"
                    )
            # Strictly consumption-ordered triggers on two engines (c 0-3 on
            # sync, c 4-7 on gpsimd).  Issuing everything up front would put
            # 30+ transfers in flight at once and starve the FIRST tile of
            # HBM bandwidth (all queues share ~358 GB/s).
            for c in range(NC_CH):  # sbb0 split in halves on two engines
                xt = xts[(0, c)]
                eng = nc.sync if c < 4 else nc.gpsimd
                eng.dma_start(xt[:, 0:512], xT[0, c, :, 0:512])
                eng.dma_start(xt[:, 512:1024], xT[0, c, :, 512:1024])
            # Strictly block-serial so block g+1 completes before g+2 starts
            # eating bandwidth -- the interleaved attention chunks depend on
            # whole blocks arriving in order.
            for sbb in (1, 2, 3):
                for c in range(NC_CH):
                    eng = nc.sync if c < 4 else nc.gpsimd
                    eng.dma_start(xts[(sbb, c)][:], xT[sbb, c])

            # ---- phase 1: projections for one 1024-col block ----
            def emit_proj_block(sbb):
                is_q = sbb < 2
                for half in range(2):
                    sb = 2 * sbb + half
                    wa = wkv_sb if half == 0 else wvk_sb
                    cols = slice(sb * 512, (sb + 1) * 512)
                    xsl = slice(half * 512, (half + 1) * 512)
                    ps_a = psum_ps.tile([128, 512], F32, tag="ps")
                    for c in range(NC_CH):
                        nc.tensor.matmul(
                            ps_a[:],
                            wa[:, c, :],
                            xts[(sbb, c)][:, xsl],
                            start=(c == 0),
                            stop=(c == NC_CH - 1),
                        )
                    nc.scalar.copy(kv2[:, cols], ps_a[:])
                    if is_q:
                        ps_b = psum_ps.tile([128, 512], F32, tag="ps")
                        for c in range(NC_CH):
                            nc.tensor.matmul(
                                ps_b[:],
                                wqq_sb[:, c, :],
                                xts[(sbb, c)][:, xsl],
                                start=(c == 0),
                                stop=(c == NC_CH - 1),
                            )
                        nc.scalar.copy(qq[:, cols], ps_b[:])
                    # V tiles of this 512-block -> va (PE transpose)
                    vrows = slice(64, 128) if half == 0 else slice(0, 64)
                    tp_pos = (64, 0) if half == 0 else (0, 0)
                    ident_sl = (
                        ident_bf[64:128, 64:128] if half == 0 else ident_bf[0:64, 0:64]
                    )
                    for j in range(4):
                        st = sb * 4 + j
                        tp = psum_ps.tile([128, 64], BF16, tag="ps")
                        nc.tensor.transpose(
                            tp[:],
                            kv2[vrows, st * 128 : (st + 1) * 128],
                            ident_sl,
                            tile_position=tp_pos,
                        )
                        nc.vector.tensor_copy(va[:, st, 0:64], tp[:])

            # key-tile pairs: (tile in even 512-block, tile in odd one)
            pairs = []
            for g in range(NSBB):
                for i in range(4):
                    pairs.append((g * 8 + i, g * 8 + 4 + i))

            # ---- phase 2 ----
            acc_tiles = {}

            def emit_scores(tcp, j):
                ta, tb = pairs[j]
                tc0 = tcp * 1024
                sc = {}
                for i in range(2):
                    qsl = slice(tc0 + i * 512, tc0 + (i + 1) * 512)
                    sc_a = psum_ps.tile(
                        [128, 512], F32, tag="ps", name=f"sca{i}_{tcp}_{j}"
                    )
                    sc_b = psum_ps.tile(
                        [128, 512], F32, tag="ps", name=f"scb{i}_{tcp}_{j}"
                    )
                    sc[("a", i)], sc[("b", i)] = sc_a, sc_b
                    nc.tensor.matmul(
                        sc_a[:],
                        kv2[0:64, ta * 128 : (ta + 1) * 128],
                        qq[0:64, qsl],
                        start=True,
                        stop=True,
                        tile_position=(0, 0),
                    )
                    nc.tensor.matmul(
                        sc_b[:],
                        kv2[64:128, tb * 128 : (tb + 1) * 128],
                        qq[64:128, qsl],
                        start=True,
                        stop=True,
                        tile_position=(64, 0),
                    )
                # exp, split per 512-chunk across engines: ACT takes the c0
                # halves (true exp), DVE the c1 halves (Schraudolph bit
                # trick) -- each query column uses one consistent method and
                # the sc psum slot is released sooner than one [128,1024]
                # pass.  Each ex tile has a SINGLE writer (Tile syncs WAW at
                # tile granularity across engines, so a shared tile would
                # ping-pong serialize ACT and DVE).
                def act_exp(sct, nm):
                    ex = exp_pool.tile([128, 512], BF16, tag="exp", name=nm)
                    nc.scalar.activation(ex[:], sct[:], EXP, scale=0.125)
                    return ex

                def dve_exp(sct, nm):
                    ex = exp_pool.tile([128, 512], BF16, tag="exp", name=nm)
                    nc.vector.tensor_scalar(
                        ex[:].bitcast(I16),
                        sct[:],
                        SCHRA_A,
                        SCHRA_B,
                        mybir.AluOpType.mult,
                        mybir.AluOpType.add,
                    )
                    return ex

                ex_a = (
                    act_exp(sc[("a", 0)], f"exa0_{tcp}_{j}"),
                    dve_exp(sc[("a", 1)], f"exa1_{tcp}_{j}"),
                )
                ex_b = (
                    act_exp(sc[("b", 0)], f"exb0_{tcp}_{j}"),
                    dve_exp(sc[("b", 1)], f"exb1_{tcp}_{j}"),
                )
                return ex_a, ex_b

            def emit_av(tcp, j, ex_a, ex_b):
                ta, tb = pairs[j]
                oc0, oc1 = acc_tiles[tcp]
                for t, ex in ((ta, ex_a), (tb, ex_b)):
                    first = j == 0 and t == ta
                    last = j == NPAIR - 1 and t == tb
                    for i, oc in enumerate((oc0, oc1)):
                        nc.tensor.matmul(
                            oc[:],
                            va[:, t, 0:65],
                            ex[i][:],
                            start=first,
                            stop=last,
                        )

            def emit_attn(tcp, j_lo, j_hi):
                if tcp not in acc_tiles:
                    acc_tiles[tcp] = (
                        psum_acc.tile([65, 512], F32, tag="acc", name=f"oc0_{tcp}"),
                        psum_acc.tile([65, 512], F32, tag="acc", name=f"oc1_{tcp}"),
                    )
                pend = []  # scores run one pair ahead of attn@V
                for j in range(j_lo, j_hi):
                    pend.append((j, emit_scores(tcp, j)))
                    if len(pend) > 1:
                        pj, (pa, pb) = pend.pop(0)
                        emit_av(tcp, pj, pa, pb)
                for pj, (pa, pb) in pend:
                    emit_av(tcp, pj, pa, pb)

            def emit_epilogue_copies(tcp):
                # frees the psum accumulators (the only thing the next query
                # group's attn@V waits on), so the rest of the epilogue can
                # trail into the next group.
                oc0, oc1 = acc_tiles[tcp]
                outt_sb = outts_pool.tile(
                    [65, 1024], F32, tag="outts", name=f"outts{tcp}"
                )
                nc.scalar.copy(outt_sb[:, 0:512], oc0[:])
                nc.scalar.copy(outt_sb[:, 512:1024], oc1[:])
                return outt_sb

            COPY = mybir.ActivationFunctionType.Copy
            out_dma_engines = (nc.sync, nc.scalar, nc.gpsimd)

            def emit_epilogue(tcp, outt_sb):
                # per 128-query block: PE transpose, DVE reciprocal of the
                # ones-column, ACT scaled-copy (out = outT.T * 1/den), DMA.
                # Normalize work is split across DVE+ACT and the out-DMA
                # triggers across three engines so no single queue serializes
                # the tail.
                for k in range(8):
                    o_ps = psum_ps.tile([128, 65], F32, tag="ps")
                    nc.tensor.transpose(
                        o_ps[:],
                        outt_sb[0:65, k * 128 : (k + 1) * 128],
                        ident_f32[0:65, 0:65],
                    )
                    rc = small_pool.tile([128, 1], F32, tag="rc")
                    nc.vector.reciprocal(rc[:], o_ps[:, 64:65])
                    o_sb = small_pool.tile([128, H], F32, tag="osb")
                    nc.scalar.activation(o_sb[:], o_ps[:, 0:H], COPY, scale=rc[:])
                    row = tcp * 1024 + k * 128
                    out_dma_engines[k % 3].dma_start(out[row : row + 128, :], o_sb[:])

            # ---- emission order ----
            # All projection work first; the attention stream is pushed
            # behind it with a manual schedule timestamp so the scheduler
            # cannot interleave late proj matmuls/transposes into the
            # (weight-pipelined) scores/attn@V streams.
            # Alternate proj blocks with tcp0 attention chunks: pair group g
            # (4 pairs) only touches block g's tiles, so each attention chunk
            # fills the DMA-wait gap of the NEXT proj block.  Emission order
            # = scheduler priority, so later proj blocks cannot preempt the
            # already-emitted attention stream.
            emit_proj_block(0)
            emit_attn(0, 0, 4)
            emit_proj_block(1)
            emit_attn(0, 4, 8)
            emit_proj_block(2)
            emit_attn(0, 8, 12)
            emit_proj_block(3)
            emit_attn(0, 12, NPAIR)
            outts0 = emit_epilogue_copies(0)
            emit_attn(1, 0, NPAIR)
            emit_epilogue(0, outts0)
            outts1 = emit_epilogue_copies(1)
            emit_epilogue(1, outts1)

    nc.compile()
    return nc


_NC_CACHE = None


def _get_module():
    global _NC_CACHE
    if _NC_CACHE is None:
        _NC_CACHE = _build_module()
    return _NC_CACHE


def _make_in_maps(x, Wq, Wk, Wv):
    bf = ml_dtypes.bfloat16
    xT = np.transpose(np.asarray(x, dtype=np.float32), (0, 2, 1))  # [B, C, T]
    wq = np.asarray(Wq, dtype=np.float32)
    wk = np.asarray(Wk, dtype=np.float32)
    wv = np.asarray(Wv, dtype=np.float32)

    def wprep(a, b):
        # [C, 128] -> sbuf layout [128 partitions, NC_CH, 128]
        w = np.concatenate([a, b], axis=1).reshape(NC_CH, 128, 128)
        return np.ascontiguousarray(w.transpose(1, 0, 2)).astype(bf)

    wkv = wprep(wk, wv)
    wvk = wprep(wv, wk)
    wqq = wprep(wq, wq)
    in_maps = []
    for core in range(N_CORES):
        b, h = divmod(core, 2)
        xt = xT[b]
        if h == 1:
            xt = np.concatenate([xt[:, TQ:], xt[:, :TQ]], axis=1)
        xt = np.ascontiguousarray(
            xt.reshape(NC_CH, 128, NSBB, 1024).transpose(2, 0, 1, 3)
        ).astype(bf)
        in_maps.append({"xT": xt, "wkv": wkv, "wvk": wvk, "wqq": wqq})
    return in_maps


def run(x, Wq, Wk, Wv, **spmd_kwargs):
    """Run on hardware; returns (output, BassKernelResults)."""
    nc = _get_module()
    in_maps = _make_in_maps(x, Wq, Wk, Wv)
    res = run_bass_kernel_spmd(nc, in_maps, core_ids=list(range(N_CORES)), **spmd_kwargs)
    out = np.empty((B, T, H), dtype=np.float32)
    for core in range(N_CORES):
        b, h = divmod(core, 2)
        out[b, h * TQ : (h + 1) * TQ, :] = res.results[core]["out"]
    return out, res


def kernel(x, Wq, Wk, Wv):
    out, _ = run(x, Wq, Wk, Wv)
    return out


# revision 36
# speedup vs baseline: 1.0495x; 1.0163x over previous
"""Single-head attention (B=4, T=4096, C=1024, H=64) on 8 trn2 NeuronCores.

Sharding: 8 shards = (batch b, query-half h).  Each core receives x[b]
pre-transposed to xT [C=1024, T=4096] in bf16; for h==1 the T columns are
rotated by 2048 so "this core's" 2048 queries are always columns 0:2048
(softmax is permutation-invariant over keys).  SPMD: identical program on
every core.

Per-core kernel (flash-attention style, all matmuls bf16):

phase 1 (projections): stream xT in [128,1024] bf16 tiles (8 c-chunks per
  1024-column block "sbb").  Two stationary streams:
    A: even 512-blocks [Wk|Wv], odd 512-blocks [Wv|Wk]  ->  kv2 [128, T]
       (K^T lives at partitions 0:64 for even blocks, 64:128 for odd ones;
        V^T on the other half) -- this feeds the row-tiled scores matmul
        with NO replication copies.
    B: [Wq|Wq] (query blocks only)  ->  qq [128, TQ]: Q^T replicated into
       both partition halves (the moving operand of both score row-tiles).
  V tiles are PE-transposed out of kv2 into va [s,64]+ones column (so the
  softmax denominator falls out of the attn@V matmul).

phase 2 (attention), per 1024-query group, per pair of key tiles (one from
  an even 512-block at partitions 0:64, one from an odd one at 64:128):
    PE: TWO CONCURRENT K=64 row-tiled matmuls (tile_position (0,0)/(64,0))
        compute scoresT [128 keys, 1024 q] for both tiles in the time of one.
    exp: tile a -> ACT true exp (bf16 out); tile b -> DVE Schraudolph
        bit-trick exp (i16 = s*a + b, bitcast bf16), splitting the exp wall
        across two engines.
    PE: outT[65, 512] += va[s,65].T @ ex  (accumulated over all 32 tiles).
  Epilogue: PE-transpose outT back to [q, 65], DVE reciprocal of the
  ones-column sum, scale, DMA out.
"""

import os
import sys

for _p in ("/opt/trn_rl_repo", "/root/.axon_site/_ro/trn_rl_repo"):
    if os.path.isdir(_p) and _p not in sys.path:
        sys.path.append(_p)

import numpy as np
import ml_dtypes

import concourse.bacc as bacc
import concourse.mybir as mybir
import concourse.tile as tile
from concourse.bass_utils import run_bass_kernel_spmd
from concourse.masks import make_identity

B = 4
T = 4096
C = 1024
H = 64
TQ = T // 2  # queries per core
N_CORES = 8

F32 = mybir.dt.float32
BF16 = mybir.dt.bfloat16
I16 = mybir.dt.int16

NC_CH = C // 128  # 8 contraction chunks
NSBB = T // 1024  # 4 1024-wide column blocks
NST = T // 128  # 32 key tiles of 128
NPAIR = NST // 2  # 16 row-tiled score pairs

EXP = mybir.ActivationFunctionType.Exp

# Schraudolph exp in bf16: i16 = trunc(z * 2^7/ln2 + (127*2^7 - c)),
# bitcast to bf16.  c = 0.0436775*128 - 0.5 (the -0.5 compensates
# truncation vs round-to-nearest).  The 0.125 score scale is folded in.
SCHRA_A = 0.125 * (2.0**7) / np.log(2.0)
SCHRA_B = 127.0 * 2.0**7 - (0.0436775 * 2.0**7 - 0.5)


def _build_module():
    nc = bacc.Bacc("TRN2", target_bir_lowering=False, debug=False, num_devices=N_CORES)

    xT = nc.dram_tensor("xT", [NSBB, NC_CH, 128, 1024], BF16, kind="ExternalInput").ap()
    wkv = nc.dram_tensor("wkv", [128, NC_CH, 128], BF16, kind="ExternalInput").ap()
    wvk = nc.dram_tensor("wvk", [128, NC_CH, 128], BF16, kind="ExternalInput").ap()
    wqq = nc.dram_tensor("wqq", [128, NC_CH, 128], BF16, kind="ExternalInput").ap()
    out = nc.dram_tensor("out", [TQ, H], F32, kind="ExternalOutput").ap()

    with tile.TileContext(nc) as tc:
        with (
            tc.tile_pool(name="const", bufs=1) as const_pool,
            tc.tile_pool(name="xt", bufs=32) as xt_pool,
            tc.tile_pool(name="big", bufs=1) as big_pool,
            tc.tile_pool(name="exp", bufs=16) as exp_pool,
            tc.tile_pool(name="outts", bufs=2) as outts_pool,
            tc.tile_pool(name="small", bufs=4) as small_pool,
            # one unified 6-slot ring (slot = 2KB/partition = 1 bank) for
            # proj / transposes / scores / epilogue; +2 banks accumulators.
            # Each score chunk gets its OWN single-bank tile: ACT and DVE can
            # only access PSUM in parallel on different banks, so a 2-bank
            # tile read by both engines would serialize them.
            tc.tile_pool(name="ps", bufs=6, space="PSUM") as psum_ps,
            tc.tile_pool(name="pacc", bufs=2, space="PSUM") as psum_acc,
        ):
            # ---- constants ----
            wkv_sb = const_pool.tile([128, NC_CH, 128], BF16, tag="wkv")
            wvk_sb = const_pool.tile([128, NC_CH, 128], BF16, tag="wvk")
            wqq_sb = const_pool.tile([128, NC_CH, 128], BF16, tag="wqq")
            ident_bf = const_pool.tile([128, 128], BF16, tag="ident_bf")
            ident_f32 = const_pool.tile([128, 128], F32, tag="ident_f32")
            # HAM warmup: junk matmuls while the x DMA streams in, so the
            # PE clock gate releases (1.2 -> 2.4 GHz) before real work starts.
            scratch = const_pool.tile([128, 512], BF16, tag="scratch")
            nc.gpsimd.memset(scratch[:], 0.0)
            for w in range(16):
                wps = psum_ps.tile([128, 512], F32, tag="ps")
                nc.tensor.matmul(
                    wps[:], scratch[:, 0:128], scratch[:], start=True, stop=True
                )

            nc.scalar.dma_start(wkv_sb[:], wkv[:])
            nc.scalar.dma_start(wvk_sb[:], wvk[:])
            nc.scalar.dma_start(wqq_sb[:], wqq[:])
            make_identity(nc, ident_bf[:])
            make_identity(nc, ident_f32[:])

            # warm the ACT exp table early (one-time ~2.7us load)
            dummy = small_pool.tile([128, 1], F32, tag="dummy")
            nc.scalar.activation(dummy[:], ident_f32[:, 0:1], EXP)

            # ---- persistent activations ----
            # kv2: even 512-block: rows 0:64 K^T, 64:128 V^T; odd: swapped
            kv2 = big_pool.tile([128, T], BF16, tag="kv2")
            qq = big_pool.tile([128, TQ], BF16, tag="qq")  # Q^T replicated
            va = big_pool.tile([128, NST, 66], BF16, tag="va")  # V[s,64]+ones
            nc.gpsimd.memset(va[:, :, 64:65], 1.0)

            # ---- x DMA: issue everything up front, 3 trigger engines ----
            xts = {}  # (sbb, c) -> tile
            for sbb in range(NSBB):
                for c in range(NC_CH):
                    xts[(sbb, c)] = xt_pool.tile(
                        [128, 1024], BF16, tag="xt", name=f"xt{sbb}_{c}"
                    )
            # Strictly consumption-ordered triggers on two engines (c 0-3 on
            # sync, c 4-7 on gpsimd).  Issuing everything up front would put
            # 30+ transfers in flight at once and starve the FIRST tile of
            # HBM bandwidth (all queues share ~358 GB/s).
            for c in range(NC_CH):  # sbb0 split in halves on two engines
                xt = xts[(0, c)]
                eng = nc.sync if c < 4 else nc.gpsimd
                eng.dma_start(xt[:, 0:512], xT[0, c, :, 0:512])
                eng.dma_start(xt[:, 512:1024], xT[0, c, :, 512:1024])
            # Strictly block-serial so block g+1 completes before g+2 starts
            # eating bandwidth -- the interleaved attention chunks depend on
            # whole blocks arriving in order.
            for sbb in (1, 2, 3):
                for c in range(NC_CH):
                    eng = nc.sync if c < 4 else nc.gpsimd
                    eng.dma_start(xts[(sbb, c)][:], xT[sbb, c])

            # ---- phase 1: projections for one 1024-col block ----
            def emit_proj_block(sbb):
                is_q = sbb < 2
                for half in range(2):
                    sb = 2 * sbb + half
                    wa = wkv_sb if half == 0 else wvk_sb
                    cols = slice(sb * 512, (sb + 1) * 512)
                    xsl = slice(half * 512, (half + 1) * 512)
                    ps_a = psum_ps.tile([128, 512], F32, tag="ps")
                    for c in range(NC_CH):
                        nc.tensor.matmul(
                            ps_a[:],
                            wa[:, c, :],
                            xts[(sbb, c)][:, xsl],
                            start=(c == 0),
                            stop=(c == NC_CH - 1),
                        )
                    nc.scalar.copy(kv2[:, cols], ps_a[:])
                    if is_q:
                        ps_b = psum_ps.tile([128, 512], F32, tag="ps")
                        for c in range(NC_CH):
                            nc.tensor.matmul(
                                ps_b[:],
                                wqq_sb[:, c, :],
                                xts[(sbb, c)][:, xsl],
                                start=(c == 0),
                                stop=(c == NC_CH - 1),
                            )
                        nc.scalar.copy(qq[:, cols], ps_b[:])
                    # V tiles of this 512-block -> va (PE transpose)
                    vrows = slice(64, 128) if half == 0 else slice(0, 64)
                    tp_pos = (64, 0) if half == 0 else (0, 0)
                    ident_sl = (
                        ident_bf[64:128, 64:128] if half == 0 else ident_bf[0:64, 0:64]
                    )
                    for j in range(4):
                        st = sb * 4 + j
                        tp = psum_ps.tile([128, 64], BF16, tag="ps")
                        nc.tensor.transpose(
                            tp[:],
                            kv2[vrows, st * 128 : (st + 1) * 128],
                            ident_sl,
                            tile_position=tp_pos,
                        )
                        nc.vector.tensor_copy(va[:, st, 0:64], tp[:])

            # key-tile pairs: (tile in even 512-block, tile in odd one)
            pairs = []
            for g in range(NSBB):
                for i in range(4):
                    pairs.append((g * 8 + i, g * 8 + 4 + i))

            # ---- phase 2 ----
            acc_tiles = {}

            def emit_scores(tcp, j):
                ta, tb = pairs[j]
                tc0 = tcp * 1024
                sc = {}
                for i in range(2):
                    qsl = slice(tc0 + i * 512, tc0 + (i + 1) * 512)
                    sc_a = psum_ps.tile(
                        [128, 512], F32, tag="ps", name=f"sca{i}_{tcp}_{j}"
                    )
                    sc_b = psum_ps.tile(
                        [128, 512], F32, tag="ps", name=f"scb{i}_{tcp}_{j}"
                    )
                    sc[("a", i)], sc[("b", i)] = sc_a, sc_b
                    nc.tensor.matmul(
                        sc_a[:],
                        kv2[0:64, ta * 128 : (ta + 1) * 128],
                        qq[0:64, qsl],
                        start=True,
                        stop=True,
                        tile_position=(0, 0),
                    )
                    nc.tensor.matmul(
                        sc_b[:],
                        kv2[64:128, tb * 128 : (tb + 1) * 128],
                        qq[64:128, qsl],
                        start=True,
                        stop=True,
                        tile_position=(64, 0),
                    )
                # exp, split per 512-chunk across engines: ACT takes the c0
                # halves (true exp), DVE the c1 halves (Schraudolph bit
                # trick) -- each query column uses one consistent method and
                # the sc psum slot is released sooner than one [128,1024]
                # pass.  Each ex tile has a SINGLE writer (Tile syncs WAW at
                # tile granularity across engines, so a shared tile would
                # ping-pong serialize ACT and DVE).
                def act_exp(sct, nm):
                    ex = exp_pool.tile([128, 512], BF16, tag="exp", name=nm)
                    nc.scalar.activation(ex[:], sct[:], EXP, scale=0.125)
                    return ex

                def dve_exp(sct, nm):
                    ex = exp_pool.tile([128, 512], BF16, tag="exp", name=nm)
                    nc.vector.tensor_scalar(
                        ex[:].bitcast(I16),
                        sct[:],
                        SCHRA_A,
                        SCHRA_B,
                        mybir.AluOpType.mult,
                        mybir.AluOpType.add,
                    )
                    return ex

                ex_a = (
                    act_exp(sc[("a", 0)], f"exa0_{tcp}_{j}"),
                    dve_exp(sc[("a", 1)], f"exa1_{tcp}_{j}"),
                )
                ex_b = (
                    act_exp(sc[("b", 0)], f"exb0_{tcp}_{j}"),
                    dve_exp(sc[("b", 1)], f"exb1_{tcp}_{j}"),
                )
                return ex_a, ex_b

            def emit_av(tcp, j, ex_a, ex_b):
                ta, tb = pairs[j]
                oc0, oc1 = acc_tiles[tcp]
                for t, ex in ((ta, ex_a), (tb, ex_b)):
                    first = j == 0 and t == ta
                    last = j == NPAIR - 1 and t == tb
                    for i, oc in enumerate((oc0, oc1)):
                        nc.tensor.matmul(
                            oc[:],
                            va[:, t, 0:65],
                            ex[i][:],
                            start=first,
                            stop=last,
                        )

            def emit_attn(tcp, j_lo, j_hi):
                if tcp not in acc_tiles:
                    acc_tiles[tcp] = (
                        psum_acc.tile([65, 512], F32, tag="acc", name=f"oc0_{tcp}"),
                        psum_acc.tile([65, 512], F32, tag="acc", name=f"oc1_{tcp}"),
                    )
                pend = []  # scores run one pair ahead of attn@V
                for j in range(j_lo, j_hi):
                    pend.append((j, emit_scores(tcp, j)))
                    if len(pend) > 1:
                        pj, (pa, pb) = pend.pop(0)
                        emit_av(tcp, pj, pa, pb)
                for pj, (pa, pb) in pend:
                    emit_av(tcp, pj, pa, pb)

            def emit_epilogue_copies(tcp):
                # frees the psum accumulators (the only thing the next query
                # group's attn@V waits on), so the rest of the epilogue can
                # trail into the next group.
                oc0, oc1 = acc_tiles[tcp]
                outt_sb = outts_pool.tile(
                    [65, 1024], F32, tag="outts", name=f"outts{tcp}"
                )
                nc.scalar.copy(outt_sb[:, 0:512], oc0[:])
                nc.scalar.copy(outt_sb[:, 512:1024], oc1[:])
                return outt_sb

            COPY = mybir.ActivationFunctionType.Copy
            out_dma_engines = (nc.sync, nc.scalar, nc.gpsimd)

            def emit_epilogue(tcp, outt_sb):
                # per 128-query block: PE transpose, DVE reciprocal of the
                # ones-column, ACT scaled-copy (out = outT.T * 1/den), DMA.
                # Normalize work is split across DVE+ACT and the out-DMA
                # triggers across three engines so no single queue serializes
                # the tail.
                for k in range(8):
                    o_ps = psum_ps.tile([128, 65], F32, tag="ps")
                    nc.tensor.transpose(
                        o_ps[:],
                        outt_sb[0:65, k * 128 : (k + 1) * 128],
                        ident_f32[0:65, 0:65],
                    )
                    rc = small_pool.tile([128, 1], F32, tag="rc")
                    nc.vector.reciprocal(rc[:], o_ps[:, 64:65])
                    o_sb = small_pool.tile([128, H], F32, tag="osb")
                    nc.scalar.activation(o_sb[:], o_ps[:, 0:H], COPY, scale=rc[:])
                    row = tcp * 1024 + k * 128
                    out_dma_engines[k % 3].dma_start(out[row : row + 128, :], o_sb[:])

            # ---- emission order ----
            # All projection work first; the attention stream is pushed
            # behind it with a manual schedule timestamp so the scheduler
            # cannot interleave late proj matmuls/transposes into the
            # (weight-pipelined) scores/attn@V streams.
            # Alternate proj blocks with tcp0 attention chunks: pair group g
            # (4 pairs) only touches block g's tiles, so each attention chunk
            # fills the DMA-wait gap of the NEXT proj block.  Emission order
            # = scheduler priority, so later proj blocks cannot preempt the
            # already-emitted attention stream.
            emit_proj_block(0)
            emit_attn(0, 0, 4)
            emit_proj_block(1)
            emit_attn(0, 4, 8)
            emit_proj_block(2)
            emit_attn(0, 8, 12)
            emit_proj_block(3)
            emit_attn(0, 12, NPAIR)
            outts0 = emit_epilogue_copies(0)
            emit_attn(1, 0, NPAIR)
            emit_epilogue(0, outts0)
            outts1 = emit_epilogue_copies(1)
            emit_epilogue(1, outts1)

    nc.compile()
    return nc


_NC_CACHE = None


def _get_module():
    global _NC_CACHE
    if _NC_CACHE is None:
        _NC_CACHE = _build_module()
    return _NC_CACHE


def _make_in_maps(x, Wq, Wk, Wv):
    bf = ml_dtypes.bfloat16
    xT = np.transpose(np.asarray(x, dtype=np.float32), (0, 2, 1))  # [B, C, T]
    wq = np.asarray(Wq, dtype=np.float32)
    wk = np.asarray(Wk, dtype=np.float32)
    wv = np.asarray(Wv, dtype=np.float32)

    def wprep(a, b):
        # [C, 128] -> sbuf layout [128 partitions, NC_CH, 128]
        w = np.concatenate([a, b], axis=1).reshape(NC_CH, 128, 128)
        return np.ascontiguousarray(w.transpose(1, 0, 2)).astype(bf)

    wkv = wprep(wk, wv)
    wvk = wprep(wv, wk)
    wqq = wprep(wq, wq)
    in_maps = []
    for core in range(N_CORES):
        b, h = divmod(core, 2)
        xt = xT[b]
        if h == 1:
            xt = np.concatenate([xt[:, TQ:], xt[:, :TQ]], axis=1)
        xt = np.ascontiguousarray(
            xt.reshape(NC_CH, 128, NSBB, 1024).transpose(2, 0, 1, 3)
        ).astype(bf)
        in_maps.append({"xT": xt, "wkv": wkv, "wvk": wvk, "wqq": wqq})
    return in_maps


def run(x, Wq, Wk, Wv, **spmd_kwargs):
    """Run on hardware; returns (output, BassKernelResults)."""
    nc = _get_module()
    in_maps = _make_in_maps(x, Wq, Wk, Wv)
    res = run_bass_kernel_spmd(nc, in_maps, core_ids=list(range(N_CORES)), **spmd_kwargs)
    out = np.empty((B, T, H), dtype=np.float32)
    for core in range(N_CORES):
        b, h = divmod(core, 2)
        out[b, h * TQ : (h + 1) * TQ, :] = res.results[core]["out"]
    return out, res


def kernel(x, Wq, Wk, Wv):
    out, _ = run(x, Wq, Wk, Wv)
    return out
